# revision 11
# baseline (speedup 1.0000x reference)
"""Trainium2 Bass kernel for the ControlledLayer problem.

Per-core work (fan_out rows sharded 8 ways, 1024 rows/core):
  ff     = W_ff_slice @ inputs          (DVE tensor_tensor_reduce matvec)
  fb     = W_fb_slice @ c               (same)
  v_new  = 0.1*v + ff + fb
  out    = sigmoid(v_new)               (ACT)
  o_spk  = out > u2                     (u2 = key-42 uniforms, host-precomputed)
  Apost' = 0.95*Apost + o_spk
  grad'  = grad + Apost' x i_spk - o_spk x Apre'   (PE rank-2 matmul + DVE add)

Host side: the jax.random key-42 uniform thresholds are data-independent, so
u1/u2 are generated on CPU; input spikes and Apre' depend only on host-known
vectors and are computed in numpy exactly as f32.
"""

import numpy as np

FAN_IN = 8192
FAN_OUT = 8192
CTRL = 8192
NCORES = 8
ROWS = FAN_OUT // NCORES  # 1024 rows per core
P = 128
NT = ROWS // P            # 8 row-tiles per core
CH = 2048                 # matvec free-dim chunk
NCH = FAN_IN // CH        # 4
GCH = 2048                # grad column chunk (DMA tile)
NGCH = FAN_IN // GCH      # 4
PCH = 1024                # psum tile columns (2 banks)

STDP_DECAY = np.float32(1.0 - 1.0 / 20)   # 0.95
ONE_MINUS_LEAK = np.float32(1.0 - 0.9)    # 0.1

_NC = None
_UNIFORMS = None


def _uniforms():
    """The spikify thresholds from reference: jax.random key 42, data-independent."""
    global _UNIFORMS
    if _UNIFORMS is None:
        import jax
        import jax.numpy as jnp

        # NOTE: this env defaults to the 'rbg' PRNG impl, whose bits are
        # backend-dependent — run on the default device exactly as the
        # reference's spikify does.
        k1, k2 = jax.random.split(jax.random.key(42))
        u1 = np.asarray(jax.random.uniform(k1, (FAN_IN,), dtype=jnp.float32))
        u2 = np.asarray(jax.random.uniform(k2, (FAN_OUT,), dtype=jnp.float32))
        _UNIFORMS = (u1, u2)
    return _UNIFORMS


def _split_multi_waits(nc, mybir, max_waits=1):
    """Walrus in this env encodes at most one sync-wait per instruction.
    Hoist extra on_wait entries onto standalone EventSemaphore insts
    inserted just before the over-subscribed instruction (same engine)."""
    import copy

    templates = {}
    for fn in nc.m.functions:
        for b in fn.blocks:
            for ins in b.instructions:
                if type(ins).__name__ == "InstEventSemaphore":
                    templates.setdefault(ins.engine, ins)
    counter = 0
    for fn in nc.m.functions:
        for b in fn.blocks:
            out = []
            for ins in b.instructions:
                si = ins.sync_info
                if si is not None and si.on_wait is not None and len(si.on_wait) > max_waits:
                    waits = list(si.on_wait)
                    tpl = templates.get(ins.engine)
                    assert tpl is not None, f"no EventSemaphore template for {ins.engine}"
                    for w in waits[:-max_waits]:
                        nop = copy.deepcopy(tpl)
                        nop.name = f"waitsplit_{counter}"
                        counter += 1
                        nop.sync_info = mybir.SyncInfo(on_wait=[w], on_update=[])
                        out.append(nop)
                    ins.sync_info = mybir.SyncInfo(
                        on_wait=waits[-max_waits:],
                        on_update=list(si.on_update or []),
                    )
                out.append(ins)
            b.instructions[:] = out


def _build_nc():
    import concourse.bass as bass
    import concourse.tile as tile
    from concourse import mybir
    from concourse.masks import make_identity

    f32 = mybir.dt.float32
    Alu = mybir.AluOpType

    nc = bass.Bass(trn_type="TRN2", target_bir_lowering=False)

    w_ff = nc.dram_tensor("w_ff", [ROWS, FAN_IN], f32, kind="ExternalInput")
    w_fb = nc.dram_tensor("w_fb", [ROWS, CTRL], f32, kind="ExternalInput")
    g_in = nc.dram_tensor("g_in", [ROWS, FAN_IN], f32, kind="ExternalInput")
    x_in = nc.dram_tensor("x_in", [FAN_IN], f32, kind="ExternalInput")
    c_in = nc.dram_tensor("c_in", [CTRL], f32, kind="ExternalInput")
    v_in = nc.dram_tensor("v_in", [ROWS], f32, kind="ExternalInput")
    apost_in = nc.dram_tensor("apost_in", [ROWS], f32, kind="ExternalInput")
    u2_in = nc.dram_tensor("u2_in", [ROWS], f32, kind="ExternalInput")
    # uv row 0 = input_spikes, row 1 = -Apre_new
    uv_in = nc.dram_tensor("uv_in", [2, FAN_IN], f32, kind="ExternalInput")

    out_o = nc.dram_tensor("out_outputs", [ROWS], f32, kind="ExternalOutput")
    out_v = nc.dram_tensor("out_v", [ROWS], f32, kind="ExternalOutput")
    out_a = nc.dram_tensor("out_apost", [ROWS], f32, kind="ExternalOutput")
    g_out = nc.dram_tensor("g_out", [ROWS, FAN_IN], f32, kind="ExternalOutput")

    def col(ap1d):
        # DRAM [ROWS] viewed as [P, NT]: row m = t*128 + p
        return ap1d[:].rearrange("(t p) -> p t", p=P)

    with tile.TileContext(nc) as tc:
        with tc.tile_pool(name="consts", bufs=1) as consts, \
             tc.tile_pool(name="wpool", bufs=5) as wpool, \
             tc.tile_pool(name="ginpool", bufs=5) as ginpool, \
             tc.tile_pool(name="psum", bufs=3, space="PSUM") as psum, \
             tc.tile_pool(name="psumt", bufs=1, space="PSUM") as psumt:

            # ---- broadcast x and c across partitions (DMA re-read) ----
            def pbcast(ap1d):
                return bass.AP(
                    tensor=ap1d.tensor,
                    offset=ap1d.offset,
                    ap=[[0, P]] + list(ap1d.ap),
                )

            xb = consts.tile([P, FAN_IN], f32)
            nc.sync.dma_start(out=xb[:], in_=pbcast(x_in[:]))
            cb = consts.tile([P, CTRL], f32)
            nc.sync.dma_start(out=cb[:], in_=pbcast(c_in[:]))

            uv_sb = consts.tile([2, FAN_IN], f32)
            nc.sync.dma_start(out=uv_sb[:], in_=uv_in[:, :])

            v_sb = consts.tile([P, NT], f32)
            nc.sync.dma_start(out=v_sb[:], in_=col(v_in))
            apost_sb = consts.tile([P, NT], f32)
            nc.sync.dma_start(out=apost_sb[:], in_=col(apost_in))
            u2_sb = consts.tile([P, NT], f32)
            nc.sync.dma_start(out=u2_sb[:], in_=col(u2_in))

            ident = consts.tile([P, P], f32)
            make_identity(nc, ident[:])

            # ---- matvecs: acc[p, t*NCH+c] = sum_f W[t*128+p, c*CH+f] * x[c*CH+f]
            acc_ff = consts.tile([P, NT * NCH], f32)
            acc_fb = consts.tile([P, NT * NCH], f32)
            for t in range(NT):
                for c in range(NCH):
                    wt = wpool.tile([P, CH], f32, tag="wt")
                    nc.sync.dma_start(
                        out=wt[:],
                        in_=w_ff[t * P:(t + 1) * P, c * CH:(c + 1) * CH],
                    )
                    j = t * NCH + c
                    nc.vector.scalar_tensor_tensor(
                        out=wt[:],
                        in0=wt[:],
                        scalar=0.0,
                        in1=xb[:, c * CH:(c + 1) * CH],
                        op0=Alu.bypass,
                        op1=Alu.mult,
                        accum_out=acc_ff[:, j:j + 1],
                    )
                for c in range(NCH):
                    wt = wpool.tile([P, CH], f32, tag="wt")
                    nc.sync.dma_start(
                        out=wt[:],
                        in_=w_fb[t * P:(t + 1) * P, c * CH:(c + 1) * CH],
                    )
                    j = t * NCH + c
                    nc.vector.scalar_tensor_tensor(
                        out=wt[:],
                        in0=wt[:],
                        scalar=0.0,
                        in1=cb[:, c * CH:(c + 1) * CH],
                        op0=Alu.bypass,
                        op1=Alu.mult,
                        accum_out=acc_fb[:, j:j + 1],
                    )

            # ---- reduce partials, leaky integrate, sigmoid, spikes ----
            ff_col = consts.tile([P, NT], f32)
            fb_col = consts.tile([P, NT], f32)
            nc.vector.tensor_reduce(
                out=ff_col[:],
                in_=acc_ff[:].rearrange("p (t c) -> p t c", c=NCH),
                axis=mybir.AxisListType.X,
                op=Alu.add,
            )
            nc.vector.tensor_reduce(
                out=fb_col[:],
                in_=acc_fb[:].rearrange("p (t c) -> p t c", c=NCH),
                axis=mybir.AxisListType.X,
                op=Alu.add,
            )
            sum_io = consts.tile([P, NT], f32)
            nc.vector.tensor_add(out=sum_io[:], in0=ff_col[:], in1=fb_col[:])
            vnew_sb = consts.tile([P, NT], f32)
            nc.vector.scalar_tensor_tensor(
                out=vnew_sb[:],
                in0=v_sb[:],
                scalar=float(ONE_MINUS_LEAK),
                in1=sum_io[:],
                op0=Alu.mult,
                op1=Alu.add,
            )
            outp_sb = consts.tile([P, NT], f32)
            nc.scalar.activation(
                out=outp_sb[:],
                in_=vnew_sb[:],
                func=mybir.ActivationFunctionType.Sigmoid,
            )
            spk_sb = consts.tile([P, NT], f32)
            nc.vector.tensor_tensor(
                out=spk_sb[:], in0=outp_sb[:], in1=u2_sb[:], op=Alu.is_gt
            )
            apostn_sb = consts.tile([P, NT], f32)
            nc.vector.scalar_tensor_tensor(
                out=apostn_sb[:],
                in0=apost_sb[:],
                scalar=float(STDP_DECAY),
                in1=spk_sb[:],
                op0=Alu.mult,
                op1=Alu.add,
            )

            nc.scalar.dma_start(out=col(out_o), in_=outp_sb[:])
            nc.scalar.dma_start(out=col(out_v), in_=vnew_sb[:])
            nc.scalar.dma_start(out=col(out_a), in_=apostn_sb[:])

            # ---- stack [apost'; o_spk] interleaved and transpose for matmul lhsT
            stack = consts.tile([P, 2 * NT], f32)
            s3 = stack[:].rearrange("p (t u) -> p t u", u=2)
            nc.vector.tensor_copy(out=s3[:, :, 0], in_=apostn_sb[:])
            nc.vector.tensor_copy(out=s3[:, :, 1], in_=spk_sb[:])

            # PE lhsT and engine reads must start at a partition-quad base, so
            # transpose each row-tile's [apost'; o_spk] pair separately.
            lhsT_tiles = []
            for t in range(NT):
                psT = psumt.tile([2, P], f32, tag="psT")
                nc.tensor.transpose(
                    out=psT[:], in_=stack[:, 2 * t:2 * t + 2], identity=ident[:]
                )
                lt = consts.tile([2, P], f32, tag=f"lhsT{t}")
                nc.scalar.copy(out=lt[:], in_=psT[:])
                lhsT_tiles.append(lt)

            # ---- grad update: g_out = g_in + lhsT[2t:2t+2].T @ uv ----
            for t in range(NT):
                for g in range(NGCH):
                    gi = ginpool.tile([P, GCH], f32, tag="gi")
                    nc.sync.dma_start(
                        out=gi[:],
                        in_=g_in[t * P:(t + 1) * P, g * GCH:(g + 1) * GCH],
                    )
                    for h in range(GCH // PCH):
                        ps = psum.tile([P, PCH], f32, tag="ps")
                        base = g * GCH + h * PCH
                        for q in range(PCH // 512):
                            nc.tensor.matmul(
                                out=ps[:, q * 512:(q + 1) * 512],
                                lhsT=lhsT_tiles[t][:],
                                rhs=uv_sb[:, base + q * 512:base + (q + 1) * 512],
                                start=True,
                                stop=True,
                            )
                        nc.vector.tensor_tensor(
                            out=gi[:, h * PCH:(h + 1) * PCH],
                            in0=gi[:, h * PCH:(h + 1) * PCH],
                            in1=ps[:],
                            op=Alu.add,
                        )
                    nc.scalar.dma_start(
                        out=g_out[t * P:(t + 1) * P, g * GCH:(g + 1) * GCH],
                        in_=gi[:],
                    )

    _split_multi_waits(nc, mybir)
    return nc


def _get_nc():
    global _NC
    if _NC is None:
        _NC = _build_nc()
    return _NC


def make_in_maps(inputs):
    x = np.ascontiguousarray(np.asarray(inputs["inputs"], dtype=np.float32))
    c = np.ascontiguousarray(np.asarray(inputs["c"], dtype=np.float32))
    v = np.asarray(inputs["v"], dtype=np.float32)
    W_ff = np.asarray(inputs["W_ff"], dtype=np.float32)
    W_fb = np.asarray(inputs["W_fb"], dtype=np.float32)
    Apre = np.asarray(inputs["Apre"], dtype=np.float32)
    Apost = np.asarray(inputs["Apost"], dtype=np.float32)
    grad_ff = np.asarray(inputs["grad_ff"], dtype=np.float32)

    u1, u2 = _uniforms()
    in_spk = (x > u1).astype(np.float32)
    apre_new = (STDP_DECAY * Apre + in_spk).astype(np.float32)
    uv = np.ascontiguousarray(np.stack([in_spk, -apre_new]).astype(np.float32))

    in_maps = []
    for i in range(NCORES):
        r0, r1 = i * ROWS, (i + 1) * ROWS
        in_maps.append({
            "w_ff": np.ascontiguousarray(W_ff[r0:r1]),
            "w_fb": np.ascontiguousarray(W_fb[r0:r1]),
            "g_in": np.ascontiguousarray(grad_ff[r0:r1]),
            "x_in": x,
            "c_in": c,
            "v_in": np.ascontiguousarray(v[r0:r1]),
            "apost_in": np.ascontiguousarray(Apost[r0:r1]),
            "u2_in": np.ascontiguousarray(u2[r0:r1]),
            "uv_in": uv,
        })
    return in_maps, apre_new


def run_device(in_maps, trace=False):
    from concourse.bass_utils import run_bass_kernel_spmd

    nc = _get_nc()
    res = run_bass_kernel_spmd(
        nc, in_maps, core_ids=list(range(NCORES)), trace=trace
    )
    return res


def kernel(**inputs):
    in_maps, apre_new = make_in_maps(inputs)
    res = run_device(in_maps, trace=False)
    outs = res.results
    outputs = np.concatenate([outs[i]["out_outputs"] for i in range(NCORES)])
    v_new = np.concatenate([outs[i]["out_v"] for i in range(NCORES)])
    apost_new = np.concatenate([outs[i]["out_apost"] for i in range(NCORES)])
    grad_new = np.concatenate([outs[i]["g_out"] for i in range(NCORES)], axis=0)
    return outputs, v_new, apre_new, apost_new, grad_new


# revision 12
# speedup vs baseline: 1.0326x; 1.0326x over previous
"""Trainium2 Bass kernel for the ControlledLayer problem.

Per-core work (fan_out rows sharded 8 ways, 1024 rows/core):
  ff     = W_ff_slice @ inputs          (DVE fused mul+reduce matvec)
  fb     = W_fb_slice @ c               (same)
  v_new  = 0.1*v + ff + fb
  out    = sigmoid(v_new)               (ACT)
  o_spk  = out > u2                     (u2 = key-42 uniforms, host-precomputed)
  Apost' = 0.95*Apost + o_spk
  grad'  = grad + Apost' x i_spk - o_spk x Apre'   (PE rank-2 matmul + DVE add)

Host side: the jax.random key-42 uniform thresholds are data-independent, so
u1/u2 are generated on CPU; input spikes and Apre' depend only on host-known
vectors and are computed in numpy exactly as f32.
"""

import numpy as np

FAN_IN = 8192
FAN_OUT = 8192
CTRL = 8192
NCORES = 8
ROWS = FAN_OUT // NCORES  # 1024 rows per core
P = 128
NT = ROWS // P            # 8 row-tiles per core
CH = 2048                 # matvec free-dim chunk
NCH = FAN_IN // CH        # 4
GCH = 2048                # grad column chunk (DMA tile)
NGCH = FAN_IN // GCH      # 4
PCH = 1024                # psum tile columns (2 banks)

STDP_DECAY = np.float32(1.0 - 1.0 / 20)   # 0.95
ONE_MINUS_LEAK = np.float32(1.0 - 0.9)    # 0.1

_NC = None
_UNIFORMS = None


def _uniforms():
    """The spikify thresholds from reference: jax.random key 42, data-independent."""
    global _UNIFORMS
    if _UNIFORMS is None:
        import jax
        import jax.numpy as jnp

        # NOTE: this env defaults to the 'rbg' PRNG impl, whose bits are
        # backend-dependent — run on the default device exactly as the
        # reference's spikify does.
        k1, k2 = jax.random.split(jax.random.key(42))
        u1 = np.asarray(jax.random.uniform(k1, (FAN_IN,), dtype=jnp.float32))
        u2 = np.asarray(jax.random.uniform(k2, (FAN_OUT,), dtype=jnp.float32))
        _UNIFORMS = (u1, u2)
    return _UNIFORMS


def _split_multi_waits(nc, mybir, max_waits=1):
    """Walrus in this env encodes at most one sync-wait per instruction.
    Hoist extra on_wait entries onto standalone EventSemaphore insts
    inserted just before the over-subscribed instruction (same engine)."""
    import copy

    templates = {}
    for fn in nc.m.functions:
        for b in fn.blocks:
            for ins in b.instructions:
                if type(ins).__name__ == "InstEventSemaphore":
                    templates.setdefault(ins.engine, ins)
    counter = 0
    for fn in nc.m.functions:
        for b in fn.blocks:
            out = []
            for ins in b.instructions:
                si = ins.sync_info
                if si is not None and si.on_wait is not None and len(si.on_wait) > max_waits:
                    waits = list(si.on_wait)
                    tpl = templates.get(ins.engine)
                    assert tpl is not None, f"no EventSemaphore template for {ins.engine}"
                    for w in waits[:-max_waits]:
                        nop = copy.deepcopy(tpl)
                        nop.name = f"waitsplit_{counter}"
                        counter += 1
                        nop.sync_info = mybir.SyncInfo(on_wait=[w], on_update=[])
                        out.append(nop)
                    ins.sync_info = mybir.SyncInfo(
                        on_wait=waits[-max_waits:],
                        on_update=list(si.on_update or []),
                    )
                out.append(ins)
            b.instructions[:] = out


def _build_nc():
    import concourse.bass as bass
    import concourse.tile as tile
    from concourse import mybir
    from concourse.masks import make_identity

    f32 = mybir.dt.float32
    Alu = mybir.AluOpType

    nc = bass.Bass(trn_type="TRN2", target_bir_lowering=False)

    w_ff = nc.dram_tensor("w_ff", [ROWS, FAN_IN], f32, kind="ExternalInput")
    w_fb = nc.dram_tensor("w_fb", [ROWS, CTRL], f32, kind="ExternalInput")
    g_in = nc.dram_tensor("g_in", [ROWS, FAN_IN], f32, kind="ExternalInput")
    x_in = nc.dram_tensor("x_in", [FAN_IN], f32, kind="ExternalInput")
    c_in = nc.dram_tensor("c_in", [CTRL], f32, kind="ExternalInput")
    v_in = nc.dram_tensor("v_in", [ROWS], f32, kind="ExternalInput")
    apost_in = nc.dram_tensor("apost_in", [ROWS], f32, kind="ExternalInput")
    u2_in = nc.dram_tensor("u2_in", [ROWS], f32, kind="ExternalInput")
    # uv row 0 = input_spikes, row 1 = -Apre_new
    uv_in = nc.dram_tensor("uv_in", [2, FAN_IN], f32, kind="ExternalInput")

    out_o = nc.dram_tensor("out_outputs", [ROWS], f32, kind="ExternalOutput")
    out_v = nc.dram_tensor("out_v", [ROWS], f32, kind="ExternalOutput")
    out_a = nc.dram_tensor("out_apost", [ROWS], f32, kind="ExternalOutput")
    g_out = nc.dram_tensor("g_out", [ROWS, FAN_IN], f32, kind="ExternalOutput")

    def col(ap1d):
        # DRAM [ROWS] viewed as [P, NT]: row m = t*128 + p
        return ap1d[:].rearrange("(t p) -> p t", p=P)

    with tile.TileContext(nc) as tc:
        with tc.tile_pool(name="consts", bufs=1) as consts, \
             tc.tile_pool(name="wpool", bufs=5) as wpool, \
             tc.tile_pool(name="ginpool", bufs=5) as ginpool, \
             tc.tile_pool(name="psum", bufs=3, space="PSUM") as psum, \
             tc.tile_pool(name="psumt", bufs=1, space="PSUM") as psumt:

            # ---- broadcast x and c across partitions (DMA re-read) ----
            def pbcast(ap1d):
                return bass.AP(
                    tensor=ap1d.tensor,
                    offset=ap1d.offset,
                    ap=[[0, P]] + list(ap1d.ap),
                )

            xb = consts.tile([P, FAN_IN], f32)
            nc.sync.dma_start(out=xb[:], in_=pbcast(x_in[:]))
            cb = consts.tile([P, CTRL], f32)
            nc.sync.dma_start(out=cb[:], in_=pbcast(c_in[:]))

            uv_sb = consts.tile([2, FAN_IN], f32)
            nc.sync.dma_start(out=uv_sb[:], in_=uv_in[:, :])

            v_sb = consts.tile([P, NT], f32)
            nc.sync.dma_start(out=v_sb[:], in_=col(v_in))
            apost_sb = consts.tile([P, NT], f32)
            nc.sync.dma_start(out=apost_sb[:], in_=col(apost_in))
            u2_sb = consts.tile([P, NT], f32)
            nc.sync.dma_start(out=u2_sb[:], in_=col(u2_in))

            ident = consts.tile([P, P], f32)
            make_identity(nc, ident[:])

            # ---- matvecs: acc[p, t*NCH+c] = sum_f W[t*128+p, c*CH+f] * x[c*CH+f]
            acc_ff = consts.tile([P, NT * NCH], f32)
            acc_fb = consts.tile([P, NT * NCH], f32)
            for t in range(NT):
                for c in range(NCH):
                    wt = wpool.tile([P, CH], f32, tag="wt")
                    nc.sync.dma_start(
                        out=wt[:],
                        in_=w_ff[t * P:(t + 1) * P, c * CH:(c + 1) * CH],
                    )
                    j = t * NCH + c
                    nc.vector.scalar_tensor_tensor(
                        out=wt[:],
                        in0=wt[:],
                        scalar=0.0,
                        in1=xb[:, c * CH:(c + 1) * CH],
                        op0=Alu.bypass,
                        op1=Alu.mult,
                        accum_out=acc_ff[:, j:j + 1],
                    )
                for c in range(NCH):
                    wt = wpool.tile([P, CH], f32, tag="wt")
                    nc.sync.dma_start(
                        out=wt[:],
                        in_=w_fb[t * P:(t + 1) * P, c * CH:(c + 1) * CH],
                    )
                    j = t * NCH + c
                    nc.vector.scalar_tensor_tensor(
                        out=wt[:],
                        in0=wt[:],
                        scalar=0.0,
                        in1=cb[:, c * CH:(c + 1) * CH],
                        op0=Alu.bypass,
                        op1=Alu.mult,
                        accum_out=acc_fb[:, j:j + 1],
                    )

            # ---- reduce partials, leaky integrate, sigmoid, spikes ----
            ff_col = consts.tile([P, NT], f32)
            fb_col = consts.tile([P, NT], f32)
            nc.vector.tensor_reduce(
                out=ff_col[:],
                in_=acc_ff[:].rearrange("p (t c) -> p t c", c=NCH),
                axis=mybir.AxisListType.X,
                op=Alu.add,
            )
            nc.vector.tensor_reduce(
                out=fb_col[:],
                in_=acc_fb[:].rearrange("p (t c) -> p t c", c=NCH),
                axis=mybir.AxisListType.X,
                op=Alu.add,
            )
            sum_io = consts.tile([P, NT], f32)
            nc.vector.tensor_add(out=sum_io[:], in0=ff_col[:], in1=fb_col[:])
            vnew_sb = consts.tile([P, NT], f32)
            nc.vector.scalar_tensor_tensor(
                out=vnew_sb[:],
                in0=v_sb[:],
                scalar=float(ONE_MINUS_LEAK),
                in1=sum_io[:],
                op0=Alu.mult,
                op1=Alu.add,
            )
            outp_sb = consts.tile([P, NT], f32)
            nc.scalar.activation(
                out=outp_sb[:],
                in_=vnew_sb[:],
                func=mybir.ActivationFunctionType.Sigmoid,
            )
            spk_sb = consts.tile([P, NT], f32)
            nc.vector.tensor_tensor(
                out=spk_sb[:], in0=outp_sb[:], in1=u2_sb[:], op=Alu.is_gt
            )
            apostn_sb = consts.tile([P, NT], f32)
            nc.vector.scalar_tensor_tensor(
                out=apostn_sb[:],
                in0=apost_sb[:],
                scalar=float(STDP_DECAY),
                in1=spk_sb[:],
                op0=Alu.mult,
                op1=Alu.add,
            )

            nc.scalar.dma_start(out=col(out_o), in_=outp_sb[:])
            nc.scalar.dma_start(out=col(out_v), in_=vnew_sb[:])
            nc.scalar.dma_start(out=col(out_a), in_=apostn_sb[:])

            # ---- stack [apost'; o_spk] interleaved and transpose for matmul lhsT
            stack = consts.tile([P, 2 * NT], f32)
            s3 = stack[:].rearrange("p (t u) -> p t u", u=2)
            nc.vector.tensor_copy(out=s3[:, :, 0], in_=apostn_sb[:])
            nc.vector.tensor_copy(out=s3[:, :, 1], in_=spk_sb[:])

            # PE lhsT and engine reads must start at a partition-quad base, so
            # transpose each row-tile's [apost'; o_spk] pair separately.
            lhsT_tiles = []
            for t in range(NT):
                psT = psumt.tile([2, P], f32, tag="psT")
                nc.tensor.transpose(
                    out=psT[:], in_=stack[:, 2 * t:2 * t + 2], identity=ident[:]
                )
                lt = consts.tile([2, P], f32, tag=f"lhsT{t}")
                nc.scalar.copy(out=lt[:], in_=psT[:])
                lhsT_tiles.append(lt)

            # ---- grad update: g_out = g_in + lhsT[2t:2t+2].T @ uv ----
            for t in range(NT):
                for g in range(NGCH):
                    gi = ginpool.tile([P, GCH], f32, tag="gi")
                    nc.sync.dma_start(
                        out=gi[:],
                        in_=g_in[t * P:(t + 1) * P, g * GCH:(g + 1) * GCH],
                    )
                    for h in range(GCH // PCH):
                        ps = psum.tile([P, PCH], f32, tag="ps")
                        base = g * GCH + h * PCH
                        for q in range(PCH // 512):
                            nc.tensor.matmul(
                                out=ps[:, q * 512:(q + 1) * 512],
                                lhsT=lhsT_tiles[t][:],
                                rhs=uv_sb[:, base + q * 512:base + (q + 1) * 512],
                                start=True,
                                stop=True,
                            )
                        nc.vector.tensor_tensor(
                            out=gi[:, h * PCH:(h + 1) * PCH],
                            in0=gi[:, h * PCH:(h + 1) * PCH],
                            in1=ps[:],
                            op=Alu.add,
                        )
                    nc.scalar.dma_start(
                        out=g_out[t * P:(t + 1) * P, g * GCH:(g + 1) * GCH],
                        in_=gi[:],
                    )

    _split_multi_waits(nc, mybir)
    return nc


def _get_nc():
    global _NC
    if _NC is None:
        _NC = _build_nc()
    return _NC


def make_in_maps(inputs):
    x = np.ascontiguousarray(np.asarray(inputs["inputs"], dtype=np.float32))
    c = np.ascontiguousarray(np.asarray(inputs["c"], dtype=np.float32))
    v = np.asarray(inputs["v"], dtype=np.float32)
    W_ff = np.asarray(inputs["W_ff"], dtype=np.float32)
    W_fb = np.asarray(inputs["W_fb"], dtype=np.float32)
    Apre = np.asarray(inputs["Apre"], dtype=np.float32)
    Apost = np.asarray(inputs["Apost"], dtype=np.float32)
    grad_ff = np.asarray(inputs["grad_ff"], dtype=np.float32)

    u1, u2 = _uniforms()
    in_spk = (x > u1).astype(np.float32)
    apre_new = (STDP_DECAY * Apre + in_spk).astype(np.float32)
    uv = np.ascontiguousarray(np.stack([in_spk, -apre_new]).astype(np.float32))

    in_maps = []
    for i in range(NCORES):
        r0, r1 = i * ROWS, (i + 1) * ROWS
        in_maps.append({
            "w_ff": np.ascontiguousarray(W_ff[r0:r1]),
            "w_fb": np.ascontiguousarray(W_fb[r0:r1]),
            "g_in": np.ascontiguousarray(grad_ff[r0:r1]),
            "x_in": x,
            "c_in": c,
            "v_in": np.ascontiguousarray(v[r0:r1]),
            "apost_in": np.ascontiguousarray(Apost[r0:r1]),
            "u2_in": np.ascontiguousarray(u2[r0:r1]),
            "uv_in": uv,
        })
    return in_maps, apre_new


def run_device(in_maps, trace=False):
    from concourse.bass_utils import run_bass_kernel_spmd

    nc = _get_nc()
    res = run_bass_kernel_spmd(
        nc, in_maps, core_ids=list(range(NCORES)), trace=trace
    )
    return res


def kernel(**inputs):
    in_maps, apre_new = make_in_maps(inputs)
    res = run_device(in_maps, trace=False)
    outs = res.results
    outputs = np.concatenate([outs[i]["out_outputs"] for i in range(NCORES)])
    v_new = np.concatenate([outs[i]["out_v"] for i in range(NCORES)])
    apost_new = np.concatenate([outs[i]["out_apost"] for i in range(NCORES)])
    grad_new = np.concatenate([outs[i]["g_out"] for i in range(NCORES)], axis=0)
    return outputs, v_new, apre_new, apost_new, grad_new
